# revision 8
# baseline (speedup 1.0000x reference)
"""Trainium2 Bass kernel for nn_MoRAttention (GQA attention with RoPE).

Reference computation (fp32):
    q = (x @ Wq.T)  -> [B,S,16,128], k/v = (x @ Wk.T/Wv.T) -> [B,S,4,128]
    rope(q), rope(k); GQA repeat kv 4x
    out = softmax(q k^T / sqrt(128)) v ; out @ Wo.T

Sharding (8 cores): core c -> (batch b = c//4, head-group g = c%4).
Each core owns q-heads [4g, 4g+4) and kv-head g (exactly one GQA group),
computes its slice of the q/k/v projections, RoPE, attention over the full
sequence, and a partial o_proj (Wo row-split).  The host sums the four
partials per batch (row-parallel unshard) and transposes back.

On-core layout is feature-major ([d, s]) so every matmul contraction sits on
the partition axis.  All matmuls run in float32r (full PE rate at N>=256,
~tf32 precision).  RoPE's rotate_half is a PE matmul with a constant
128x128 rotation matrix (DVE cannot cross partitions).  Softmax skips the
max-subtraction (scores are O(5) for N(0,1) inputs; exp is computed in fp32)
and gets the denominator from an accumulated ones-matmul.
"""

import math
import os

import numpy as np

import concourse.bass as bass
import concourse.mybir as mybir
import concourse.tile as tile
from concourse import bacc
from concourse.bass_utils import run_bass_kernel_spmd
from concourse.masks import make_identity

B, S, H = 2, 2048, 2048
NH, NKV, DH = 16, 4, 128
NCORES = 8
TPG = 4  # cores per batch (tensor-parallel on heads)
HPC = NH // TPG  # q heads per core = 4
QH = HPC * DH  # per-core q width = 512
SCALE = 1.0 / math.sqrt(DH)
ROPE_THETA = 10000.0

KT = H // 128  # 16 contraction tiles over the model dim
NCHUNK = 4  # seq chunks
CS = S // NCHUNK  # 512
ST = S // 128  # 16 seq tiles

F32 = mybir.dt.float32
F32R = mybir.dt.float32r

Exp = mybir.ActivationFunctionType.Exp
Copy = mybir.ActivationFunctionType.Copy
ADD = mybir.AluOpType.add
MULT = mybir.AluOpType.mult


def _emit(nc, tc, ctx):
    import contextlib

    hT = nc.dram_tensor("hT", [H, S], F32R, kind="ExternalInput")
    wqT = nc.dram_tensor("wqT", [H, QH], F32R, kind="ExternalInput")
    wkT = nc.dram_tensor("wkT", [H, DH], F32R, kind="ExternalInput")
    wvT = nc.dram_tensor("wvT", [H, DH], F32R, kind="ExternalInput")
    woT = nc.dram_tensor("woT", [QH, H], F32R, kind="ExternalInput")
    cosT = nc.dram_tensor("cosT", [DH, S], F32, kind="ExternalInput")
    sinT = nc.dram_tensor("sinT", [DH, S], F32, kind="ExternalInput")
    rotT = nc.dram_tensor("rotT", [DH, DH], F32R, kind="ExternalInput")
    ones = nc.dram_tensor("ones", [128, 1], F32R, kind="ExternalInput")
    outT = nc.dram_tensor("outT", [H, S], F32, kind="ExternalOutput")

    const = ctx.enter_context(tc.tile_pool(name="const", bufs=1))

    # Resident weights, [128, kt, m] so lhsT slices are [128, 128-ish]
    wq_sb = const.tile([128, KT, QH], F32R)
    nc.sync.dma_start(out=wq_sb[:], in_=wqT.rearrange("(t p) m -> p t m", p=128))
    wk_sb = const.tile([128, KT, DH], F32R)
    nc.sync.dma_start(out=wk_sb[:], in_=wkT.rearrange("(t p) m -> p t m", p=128))
    wv_sb = const.tile([128, KT, DH], F32R)
    nc.sync.dma_start(out=wv_sb[:], in_=wvT.rearrange("(t p) m -> p t m", p=128))
    wo_sb = const.tile([128, HPC, H], F32R)
    nc.sync.dma_start(out=wo_sb[:], in_=woT.rearrange("(h p) m -> p h m", p=128))
    cos_sb = const.tile([DH, S], F32)
    nc.sync.dma_start(out=cos_sb[:], in_=cosT[:])
    sin_sb = const.tile([DH, S], F32)
    nc.sync.dma_start(out=sin_sb[:], in_=sinT[:])
    rot_sb = const.tile([DH, DH], F32R)
    nc.sync.dma_start(out=rot_sb[:], in_=rotT[:])
    ones_sb = const.tile([128, 1], F32R)
    nc.sync.dma_start(out=ones_sb[:], in_=ones[:])
    ident = const.tile([128, 128], F32)
    make_identity(nc, ident[:])

    # Per-core persistent activations (feature-major)
    q_rope = const.tile([128, HPC, S], F32R)  # rope'd q heads, [d, h, s]
    k_rope = const.tile([128, S], F32R)  # rope'd k, [d, s]
    v_sb = const.tile([128, ST, DH], F32R)  # v, [s-tile part, st, d]

    hT_t = hT.rearrange("(t p) s -> t p s", p=128)

    # ---------------- Phase A: projections ----------------
    with (
        tc.tile_pool(name="hload", bufs=3) as hload,
        tc.tile_pool(name="qps", bufs=1, space="PSUM") as qps,
        tc.tile_pool(name="kvps", bufs=1, space="PSUM") as kvps,
        tc.tile_pool(name="miscps", bufs=1, space="PSUM") as miscps,
        tc.tile_pool(name="evac", bufs=2) as evac,
        tc.tile_pool(name="ropetmp", bufs=2) as ropetmp,
    ):
        for c in range(NCHUNK):
            sl = bass.ts(c, CS)  # this chunk's seq slice
            q_ps = [
                qps.tile([128, CS], F32, tag=f"q{h}", name=f"q_ps{h}", bufs=1)
                for h in range(HPC)
            ]
            k_ps = kvps.tile([128, CS], F32, tag="k")
            vT_ps = kvps.tile([128, CS], F32, tag="v")
            for kt in range(KT):
                h_tile = hload.tile([128, CS], F32R)
                nc.sync.dma_start(out=h_tile[:], in_=hT_t[kt, :, sl])
                mm = dict(start=(kt == 0), stop=(kt == KT - 1))
                for h in range(HPC):
                    nc.tensor.matmul(
                        q_ps[h][:], wq_sb[:, kt, bass.ts(h, DH)], h_tile[:], **mm
                    )
                nc.tensor.matmul(k_ps[:], wk_sb[:, kt, :], h_tile[:], **mm)
                nc.tensor.matmul(vT_ps[:], wv_sb[:, kt, :], h_tile[:], **mm)

            # V: evacuate vT ([d, s]) then PE-transpose into [s, d]
            vT_sb = evac.tile([128, CS], F32, tag="vT")
            nc.scalar.copy(out=vT_sb[:], in_=vT_ps[:])
            vtr_ps = miscps.tile([128, CS], F32, tag="vtr")
            for i in range(CS // 128):
                nc.tensor.transpose(
                    vtr_ps[:, bass.ts(i, 128)], vT_sb[:, bass.ts(i, 128)], ident[:]
                )
            nc.scalar.copy(
                out=v_sb[:, 4 * c : 4 * (c + 1), :].rearrange("p a b -> p (a b)"),
                in_=vtr_ps[:],
            )

            # Q/K: evacuate, rotate via PE, rope via DVE
            for h in range(HPC + 1):
                if h < HPC:
                    src_ps = q_ps[h]
                    dst = q_rope[:, h, sl]
                else:
                    src_ps = k_ps
                    dst = k_rope[:, sl]
                f_sb = evac.tile([128, CS], F32R, tag="f")
                nc.scalar.copy(out=f_sb[:], in_=src_ps[:])
                r_ps = miscps.tile([128, CS], F32, tag="rot")
                nc.tensor.matmul(r_ps[:], rot_sb[:], f_sb[:], start=True, stop=True)
                t1 = ropetmp.tile([128, CS], F32, tag="t1")
                nc.vector.tensor_tensor(t1[:], f_sb[:], cos_sb[:, sl], op=MULT)
                t2 = ropetmp.tile([128, CS], F32, tag="t2")
                nc.vector.tensor_tensor(t2[:], r_ps[:], sin_sb[:, sl], op=MULT)
                nc.vector.tensor_tensor(dst, t1[:], t2[:], op=ADD)

    # ---------------- Phase B+C: attention + o_proj ----------------
    PAIR = 2  # score k-tiles exp'd per ACT op (psum banks per scores tile)
    with (
        tc.tile_pool(name="sps", bufs=2, space="PSUM") as sps,
        tc.tile_pool(name="ops", bufs=2, space="PSUM") as ops,
        tc.tile_pool(name="dps", bufs=1, space="PSUM") as dps,
        tc.tile_pool(name="cps", bufs=1, space="PSUM") as cps,
        tc.tile_pool(name="expp", bufs=3) as expp,
        tc.tile_pool(name="opool", bufs=2) as opool,
        tc.tile_pool(name="small", bufs=2) as small,
        tc.tile_pool(name="outev", bufs=3) as outev,
        tc.tile_pool(name="drb", bufs=2, space="DRAM") as drb,
    ):
        for c in range(NCHUNK):
            sl = bass.ts(c, CS)
            o_chunk = opool.tile([128, HPC, CS], F32R)
            for h in range(HPC):
                o_ps = ops.tile([128, CS], F32)
                d_ps = dps.tile([1, CS], F32)
                for kp in range(ST // PAIR):
                    s_ps = sps.tile([128, PAIR * CS], F32)
                    for j in range(PAIR):
                        kt = kp * PAIR + j
                        nc.tensor.matmul(
                            s_ps[:, bass.ts(j, CS)],
                            k_rope[:, bass.ts(kt, 128)],
                            q_rope[:, h, sl],
                            start=True,
                            stop=True,
                        )
                    e_sb = expp.tile([128, PAIR * CS], F32R)
                    nc.scalar.activation(e_sb[:], s_ps[:], Exp, scale=SCALE)
                    for j in range(PAIR):
                        kt = kp * PAIR + j
                        mm = dict(start=(kt == 0), stop=(kt == ST - 1))
                        nc.tensor.matmul(
                            o_ps[:], v_sb[:, kt, :], e_sb[:, bass.ts(j, CS)], **mm
                        )
                        nc.tensor.matmul(
                            d_ps[:], ones_sb[:], e_sb[:, bass.ts(j, CS)], **mm
                        )
                # normalize: o / denom
                recip = small.tile([1, CS], F32, tag="recip")
                nc.vector.reciprocal_approx_fast(recip[:], d_ps[:])
                r_dram = drb.tile([1, CS], F32)
                nc.sync.dma_start(out=r_dram[:], in_=recip[:])
                recip_bc = small.tile([128, CS], F32, tag="rbc")
                nc.sync.dma_start(
                    out=recip_bc[:], in_=r_dram[:].partition_broadcast(128)
                )
                nc.vector.tensor_tensor(
                    o_chunk[:, h, :], o_ps[:], recip_bc[:], op=MULT
                )
            # o_proj partial for this chunk
            for mt in range(KT):
                c_ps = cps.tile([128, CS], F32)
                for h in range(HPC):
                    nc.tensor.matmul(
                        c_ps[:],
                        wo_sb[:, h, bass.ts(mt, 128)],
                        o_chunk[:, h, :],
                        start=(h == 0),
                        stop=(h == HPC - 1),
                    )
                o_ev = outev.tile([128, CS], F32)
                nc.scalar.copy(out=o_ev[:], in_=c_ps[:])
                nc.sync.dma_start(out=outT[bass.ts(mt, 128), sl], in_=o_ev[:])


def build():
    nc = bacc.Bacc("TRN2", target_bir_lowering=False)
    import contextlib

    with tile.TileContext(nc) as tc:
        with contextlib.ExitStack() as ctx:
            _emit(nc, tc, ctx)
    nc.compile()
    return nc


_NC = None


def _get_nc():
    global _NC
    if _NC is None:
        _NC = build()
    return _NC


def _host_tables():
    inv = 1.0 / (ROPE_THETA ** (np.arange(0, DH, 2, dtype=np.float64) / DH))
    t = np.arange(S, dtype=np.float64)
    freqs = np.outer(t, inv)  # [S, 64]
    emb = np.concatenate([freqs, freqs], axis=1)  # [S, 128]
    cosT = np.ascontiguousarray(np.cos(emb).T.astype(np.float32))  # [128, S]
    sinT = np.ascontiguousarray(np.sin(emb).T.astype(np.float32))
    # rot[d,:] selects rotate_half: rot @ q = concat(-q_hi, q_lo)
    half = DH // 2
    rot = np.zeros((DH, DH), np.float32)
    for d in range(half):
        rot[d, d + half] = -1.0
        rot[d + half, d] = 1.0
    rotT = np.ascontiguousarray(rot.T)
    return cosT, sinT, rotT


LAST_EXEC_TIME_NS = None
LAST_TRACE = None


def kernel(hidden_states, attention_mask, Wq, Wk, Wv, Wo):
    global LAST_EXEC_TIME_NS, LAST_TRACE
    hidden_states = np.asarray(hidden_states, dtype=np.float32)
    Wq = np.asarray(Wq, dtype=np.float32)
    Wk = np.asarray(Wk, dtype=np.float32)
    Wv = np.asarray(Wv, dtype=np.float32)
    Wo = np.asarray(Wo, dtype=np.float32)

    nc = _get_nc()
    cosT, sinT, rotT = _host_tables()
    ones = np.ones((128, 1), np.float32)

    hTs = [np.ascontiguousarray(hidden_states[b].T) for b in range(B)]
    in_maps = []
    for core in range(NCORES):
        b, g = divmod(core, TPG)
        qsl = slice(g * QH, (g + 1) * QH)
        ksl = slice(g * DH, (g + 1) * DH)
        in_maps.append(
            {
                "hT": hTs[b],
                "wqT": np.ascontiguousarray(Wq[qsl].T),
                "wkT": np.ascontiguousarray(Wk[ksl].T),
                "wvT": np.ascontiguousarray(Wv[ksl].T),
                "woT": np.ascontiguousarray(Wo[:, qsl].T),
                "cosT": cosT,
                "sinT": sinT,
                "rotT": rotT,
                "ones": ones,
            }
        )

    trace = bool(os.environ.get("BASS_KERNEL_TRACE"))
    kw = {}
    if trace:
        kw = dict(trace=True, trace_cores=list(range(NCORES)))
    res = run_bass_kernel_spmd(nc, in_maps, core_ids=list(range(NCORES)), **kw)
    LAST_EXEC_TIME_NS = res.exec_time_ns
    LAST_TRACE = res.instructions_and_trace[1] if res.instructions_and_trace else None

    out = np.zeros((B, H, S), np.float32)
    for core in range(NCORES):
        out[core // TPG] += res.results[core]["outT"]
    return np.ascontiguousarray(out.transpose(0, 2, 1))


# revision 9
# speedup vs baseline: 1.0258x; 1.0258x over previous
"""Trainium2 Bass kernel for nn_MoRAttention (GQA attention with RoPE).

Reference computation (fp32):
    q = (x @ Wq.T)  -> [B,S,16,128], k/v = (x @ Wk.T/Wv.T) -> [B,S,4,128]
    rope(q), rope(k); GQA repeat kv 4x
    out = softmax(q k^T / sqrt(128)) v ; out @ Wo.T

Sharding (8 cores): core c -> (batch b = c//4, head-group g = c%4).
Each core owns q-heads [4g, 4g+4) and kv-head g (exactly one GQA group),
computes its slice of the q/k/v projections, RoPE, attention over the full
sequence, and a partial o_proj (Wo row-split).  The host sums the four
partials per batch (row-parallel unshard) and transposes back.

On-core layout is feature-major ([d, s]) so every matmul contraction sits on
the partition axis.  All matmuls run in float32r (full PE rate at N>=256,
~tf32 precision).  RoPE's rotate_half is a PE matmul with a constant
128x128 rotation matrix (DVE cannot cross partitions).  Softmax skips the
max-subtraction (scores are O(5) for N(0,1) inputs; exp is computed in fp32)
and gets the denominator from an accumulated ones-matmul.
"""

import math
import os

import numpy as np

import concourse.bass as bass
import concourse.mybir as mybir
import concourse.tile as tile
from concourse import bacc
from concourse.bass_utils import run_bass_kernel_spmd
from concourse.masks import make_identity

B, S, H = 2, 2048, 2048
NH, NKV, DH = 16, 4, 128
NCORES = 8
TPG = 4  # cores per batch (tensor-parallel on heads)
HPC = NH // TPG  # q heads per core = 4
QH = HPC * DH  # per-core q width = 512
SCALE = 1.0 / math.sqrt(DH)
ROPE_THETA = 10000.0

KT = H // 128  # 16 contraction tiles over the model dim
NCHUNK = 4  # seq chunks
CS = S // NCHUNK  # 512
ST = S // 128  # 16 seq tiles

F32 = mybir.dt.float32
F32R = mybir.dt.float32r
BF16 = mybir.dt.bfloat16
COMPUTE_DT = os.environ.get("ATTN_COMPUTE_DT", "bf16")
MMD = {"bf16": BF16, "f32r": F32R}[COMPUTE_DT]

Exp = mybir.ActivationFunctionType.Exp
Copy = mybir.ActivationFunctionType.Copy
ADD = mybir.AluOpType.add
MULT = mybir.AluOpType.mult


def _emit(nc, tc, ctx):
    import contextlib

    hT = nc.dram_tensor("hT", [H, S], MMD, kind="ExternalInput")
    wqT = nc.dram_tensor("wqT", [H, QH], MMD, kind="ExternalInput")
    wkT = nc.dram_tensor("wkT", [H, DH], MMD, kind="ExternalInput")
    wvT = nc.dram_tensor("wvT", [H, DH], MMD, kind="ExternalInput")
    woT = nc.dram_tensor("woT", [QH, H], MMD, kind="ExternalInput")
    cosT = nc.dram_tensor("cosT", [DH, S], F32, kind="ExternalInput")
    sinT = nc.dram_tensor("sinT", [DH, S], F32, kind="ExternalInput")
    rotT = nc.dram_tensor("rotT", [DH, DH], MMD, kind="ExternalInput")
    ones = nc.dram_tensor("ones", [128, 1], MMD, kind="ExternalInput")
    outT = nc.dram_tensor("outT", [H, S], F32, kind="ExternalOutput")

    const = ctx.enter_context(tc.tile_pool(name="const", bufs=1))

    # Resident weights, [128, kt, m] so lhsT slices are [128, 128-ish]
    wq_sb = const.tile([128, KT, QH], MMD)
    nc.sync.dma_start(out=wq_sb[:], in_=wqT.rearrange("(t p) m -> p t m", p=128))
    wk_sb = const.tile([128, KT, DH], MMD)
    nc.sync.dma_start(out=wk_sb[:], in_=wkT.rearrange("(t p) m -> p t m", p=128))
    wv_sb = const.tile([128, KT, DH], MMD)
    nc.sync.dma_start(out=wv_sb[:], in_=wvT.rearrange("(t p) m -> p t m", p=128))
    wo_sb = const.tile([128, HPC, H], MMD)
    nc.sync.dma_start(out=wo_sb[:], in_=woT.rearrange("(h p) m -> p h m", p=128))
    cos_sb = const.tile([DH, S], F32)
    nc.sync.dma_start(out=cos_sb[:], in_=cosT[:])
    sin_sb = const.tile([DH, S], F32)
    nc.sync.dma_start(out=sin_sb[:], in_=sinT[:])
    rot_sb = const.tile([DH, DH], MMD)
    nc.sync.dma_start(out=rot_sb[:], in_=rotT[:])
    ones_sb = const.tile([128, 1], MMD)
    nc.sync.dma_start(out=ones_sb[:], in_=ones[:])
    ident = const.tile([128, 128], F32)
    make_identity(nc, ident[:])

    # Per-core persistent activations (feature-major)
    q_rope = const.tile([128, HPC, S], MMD)  # rope'd q heads, [d, h, s]
    k_rope = const.tile([128, S], MMD)  # rope'd k, [d, s]
    v_sb = const.tile([128, ST, DH], MMD)  # v, [s-tile part, st, d]

    hT_t = hT.rearrange("(t p) s -> t p s", p=128)

    # ---------------- Phase A: projections ----------------
    with (
        tc.tile_pool(name="hload", bufs=3) as hload,
        tc.tile_pool(name="qps", bufs=1, space="PSUM") as qps,
        tc.tile_pool(name="kvps", bufs=1, space="PSUM") as kvps,
        tc.tile_pool(name="miscps", bufs=1, space="PSUM") as miscps,
        tc.tile_pool(name="evac", bufs=2) as evac,
        tc.tile_pool(name="ropetmp", bufs=2) as ropetmp,
    ):
        for c in range(NCHUNK):
            sl = bass.ts(c, CS)  # this chunk's seq slice
            q_ps = [
                qps.tile([128, CS], F32, tag=f"q{h}", name=f"q_ps{h}", bufs=1)
                for h in range(HPC)
            ]
            k_ps = kvps.tile([128, CS], F32, tag="k")
            vT_ps = kvps.tile([128, CS], F32, tag="v")
            for kt in range(KT):
                h_tile = hload.tile([128, CS], MMD)
                nc.sync.dma_start(out=h_tile[:], in_=hT_t[kt, :, sl])
                mm = dict(start=(kt == 0), stop=(kt == KT - 1))
                for h in range(HPC):
                    nc.tensor.matmul(
                        q_ps[h][:], wq_sb[:, kt, bass.ts(h, DH)], h_tile[:], **mm
                    )
                nc.tensor.matmul(k_ps[:], wk_sb[:, kt, :], h_tile[:], **mm)
                nc.tensor.matmul(vT_ps[:], wv_sb[:, kt, :], h_tile[:], **mm)

            # V: evacuate vT ([d, s]) then PE-transpose into [s, d]
            vT_sb = evac.tile([128, CS], F32, tag="vT")
            nc.scalar.copy(out=vT_sb[:], in_=vT_ps[:])
            vtr_ps = miscps.tile([128, CS], F32, tag="vtr")
            for i in range(CS // 128):
                nc.tensor.transpose(
                    vtr_ps[:, bass.ts(i, 128)], vT_sb[:, bass.ts(i, 128)], ident[:]
                )
            nc.scalar.copy(
                out=v_sb[:, 4 * c : 4 * (c + 1), :].rearrange("p a b -> p (a b)"),
                in_=vtr_ps[:],
            )

            # Q/K: evacuate, rotate via PE, rope via DVE
            for h in range(HPC + 1):
                if h < HPC:
                    src_ps = q_ps[h]
                    dst = q_rope[:, h, sl]
                else:
                    src_ps = k_ps
                    dst = k_rope[:, sl]
                f_sb = evac.tile([128, CS], MMD, tag="f")
                nc.scalar.copy(out=f_sb[:], in_=src_ps[:])
                r_ps = miscps.tile([128, CS], F32, tag="rot")
                nc.tensor.matmul(r_ps[:], rot_sb[:], f_sb[:], start=True, stop=True)
                t1 = ropetmp.tile([128, CS], F32, tag="t1")
                nc.vector.tensor_tensor(t1[:], f_sb[:], cos_sb[:, sl], op=MULT)
                t2 = ropetmp.tile([128, CS], F32, tag="t2")
                nc.vector.tensor_tensor(t2[:], r_ps[:], sin_sb[:, sl], op=MULT)
                nc.vector.tensor_tensor(dst, t1[:], t2[:], op=ADD)

    # ---------------- Phase B+C: attention + o_proj ----------------
    PAIR = 2  # score k-tiles exp'd per ACT op (psum banks per scores tile)
    with (
        tc.tile_pool(name="sps", bufs=2, space="PSUM") as sps,
        tc.tile_pool(name="ops", bufs=2, space="PSUM") as ops,
        tc.tile_pool(name="dps", bufs=1, space="PSUM") as dps,
        tc.tile_pool(name="cps", bufs=1, space="PSUM") as cps,
        tc.tile_pool(name="expp", bufs=3) as expp,
        tc.tile_pool(name="opool", bufs=2) as opool,
        tc.tile_pool(name="small", bufs=2) as small,
        tc.tile_pool(name="outev", bufs=3) as outev,
        tc.tile_pool(name="drb", bufs=2, space="DRAM") as drb,
    ):
        for c in range(NCHUNK):
            sl = bass.ts(c, CS)
            o_chunk = opool.tile([128, HPC, CS], MMD)
            for h in range(HPC):
                o_ps = ops.tile([128, CS], F32)
                d_ps = dps.tile([1, CS], F32)
                for kp in range(ST // PAIR):
                    s_ps = sps.tile([128, PAIR * CS], F32)
                    for j in range(PAIR):
                        kt = kp * PAIR + j
                        nc.tensor.matmul(
                            s_ps[:, bass.ts(j, CS)],
                            k_rope[:, bass.ts(kt, 128)],
                            q_rope[:, h, sl],
                            start=True,
                            stop=True,
                        )
                    e_sb = expp.tile([128, PAIR * CS], MMD)
                    nc.scalar.activation(e_sb[:], s_ps[:], Exp, scale=SCALE)
                    for j in range(PAIR):
                        kt = kp * PAIR + j
                        mm = dict(start=(kt == 0), stop=(kt == ST - 1))
                        nc.tensor.matmul(
                            o_ps[:], v_sb[:, kt, :], e_sb[:, bass.ts(j, CS)], **mm
                        )
                        nc.tensor.matmul(
                            d_ps[:], ones_sb[:], e_sb[:, bass.ts(j, CS)], **mm
                        )
                # normalize: o / denom
                recip = small.tile([1, CS], F32, tag="recip")
                nc.vector.reciprocal_approx_fast(recip[:], d_ps[:])
                r_dram = drb.tile([1, CS], F32)
                nc.sync.dma_start(out=r_dram[:], in_=recip[:])
                recip_bc = small.tile([128, CS], F32, tag="rbc")
                nc.sync.dma_start(
                    out=recip_bc[:], in_=r_dram[:].partition_broadcast(128)
                )
                nc.vector.tensor_tensor(
                    o_chunk[:, h, :], o_ps[:], recip_bc[:], op=MULT
                )
            # o_proj partial for this chunk
            for mt in range(KT):
                c_ps = cps.tile([128, CS], F32)
                for h in range(HPC):
                    nc.tensor.matmul(
                        c_ps[:],
                        wo_sb[:, h, bass.ts(mt, 128)],
                        o_chunk[:, h, :],
                        start=(h == 0),
                        stop=(h == HPC - 1),
                    )
                o_ev = outev.tile([128, CS], F32)
                nc.scalar.copy(out=o_ev[:], in_=c_ps[:])
                nc.sync.dma_start(out=outT[bass.ts(mt, 128), sl], in_=o_ev[:])


def build():
    nc = bacc.Bacc("TRN2", target_bir_lowering=False)
    import contextlib

    with tile.TileContext(nc) as tc:
        with contextlib.ExitStack() as ctx:
            _emit(nc, tc, ctx)
    nc.compile()
    return nc


_NC = None


def _get_nc():
    global _NC
    if _NC is None:
        _NC = build()
    return _NC


def _host_tables():
    inv = 1.0 / (ROPE_THETA ** (np.arange(0, DH, 2, dtype=np.float64) / DH))
    t = np.arange(S, dtype=np.float64)
    freqs = np.outer(t, inv)  # [S, 64]
    emb = np.concatenate([freqs, freqs], axis=1)  # [S, 128]
    cosT = np.ascontiguousarray(np.cos(emb).T.astype(np.float32))  # [128, S]
    sinT = np.ascontiguousarray(np.sin(emb).T.astype(np.float32))
    # rot[d,:] selects rotate_half: rot @ q = concat(-q_hi, q_lo)
    half = DH // 2
    rot = np.zeros((DH, DH), np.float32)
    for d in range(half):
        rot[d, d + half] = -1.0
        rot[d + half, d] = 1.0
    rotT = np.ascontiguousarray(rot.T)
    return cosT, sinT, rotT


LAST_EXEC_TIME_NS = None
LAST_TRACE = None


def _mmd_np(a):
    if COMPUTE_DT == "bf16":
        import ml_dtypes

        return np.ascontiguousarray(a.astype(ml_dtypes.bfloat16))
    return np.ascontiguousarray(a.astype(np.float32))


def kernel(hidden_states, attention_mask, Wq, Wk, Wv, Wo):
    global LAST_EXEC_TIME_NS, LAST_TRACE
    hidden_states = np.asarray(hidden_states, dtype=np.float32)
    Wq = np.asarray(Wq, dtype=np.float32)
    Wk = np.asarray(Wk, dtype=np.float32)
    Wv = np.asarray(Wv, dtype=np.float32)
    Wo = np.asarray(Wo, dtype=np.float32)

    nc = _get_nc()
    cosT, sinT, rotT = _host_tables()
    ones = np.ones((128, 1), np.float32)

    hTs = [_mmd_np(hidden_states[b].T) for b in range(B)]
    in_maps = []
    for core in range(NCORES):
        b, g = divmod(core, TPG)
        qsl = slice(g * QH, (g + 1) * QH)
        ksl = slice(g * DH, (g + 1) * DH)
        in_maps.append(
            {
                "hT": hTs[b],
                "wqT": _mmd_np(Wq[qsl].T),
                "wkT": _mmd_np(Wk[ksl].T),
                "wvT": _mmd_np(Wv[ksl].T),
                "woT": _mmd_np(Wo[:, qsl].T),
                "cosT": cosT,
                "sinT": sinT,
                "rotT": _mmd_np(rotT),
                "ones": _mmd_np(ones),
            }
        )

    trace = bool(os.environ.get("BASS_KERNEL_TRACE"))
    kw = {}
    if trace:
        kw = dict(trace=True, trace_cores=list(range(NCORES)))
    res = run_bass_kernel_spmd(nc, in_maps, core_ids=list(range(NCORES)), **kw)
    LAST_EXEC_TIME_NS = res.exec_time_ns
    LAST_TRACE = res.instructions_and_trace[1] if res.instructions_and_trace else None

    out = np.zeros((B, H, S), np.float32)
    for core in range(NCORES):
        out[core // TPG] += res.results[core]["outT"]
    return np.ascontiguousarray(out.transpose(0, 2, 1))


# revision 12
# speedup vs baseline: 1.2700x; 1.2380x over previous
"""Trainium2 Bass kernel for nn_MoRAttention (GQA attention with RoPE).

Reference computation (fp32):
    q = (x @ Wq.T)  -> [B,S,16,128], k/v = (x @ Wk.T/Wv.T) -> [B,S,4,128]
    rope(q), rope(k); GQA repeat kv 4x
    out = softmax(q k^T / sqrt(128)) v ; out @ Wo.T

Sharding (8 cores): core c -> (batch b = c//4, head-group g = c%4).
Each core owns q-heads [4g, 4g+4) and kv-head g (exactly one GQA group),
computes its slice of the q/k/v projections, RoPE, attention over the full
sequence, and a partial o_proj (Wo row-split).  The host sums the four
partials per batch (row-parallel unshard) and transposes back.

On-core layout is feature-major ([d, s]) so every matmul contraction sits on
the partition axis.  All matmuls run in float32r (full PE rate at N>=256,
~tf32 precision).  RoPE's rotate_half is a PE matmul with a constant
128x128 rotation matrix (DVE cannot cross partitions).  Softmax skips the
max-subtraction (scores are O(5) for N(0,1) inputs; exp is computed in fp32)
and gets the denominator from an accumulated ones-matmul.
"""

import math
import os

import numpy as np

import concourse.bass as bass
import concourse.mybir as mybir
import concourse.tile as tile
from concourse import bacc
from concourse.bass_utils import run_bass_kernel_spmd
from concourse.masks import make_identity

B, S, H = 2, 2048, 2048
NH, NKV, DH = 16, 4, 128
NCORES = 8
TPG = 4  # cores per batch (tensor-parallel on heads)
HPC = NH // TPG  # q heads per core = 4
QH = HPC * DH  # per-core q width = 512
SCALE = 1.0 / math.sqrt(DH)
ROPE_THETA = 10000.0

KT = H // 128  # 16 contraction tiles over the model dim
NCHUNK = 4  # seq chunks
CS = S // NCHUNK  # 512
ST = S // 128  # 16 seq tiles

F32 = mybir.dt.float32
F32R = mybir.dt.float32r
BF16 = mybir.dt.bfloat16
COMPUTE_DT = os.environ.get("ATTN_COMPUTE_DT", "bf16")
MMD = {"bf16": BF16, "f32r": F32R}[COMPUTE_DT]

Exp = mybir.ActivationFunctionType.Exp
Copy = mybir.ActivationFunctionType.Copy
ADD = mybir.AluOpType.add
MULT = mybir.AluOpType.mult


def _emit(nc, tc, ctx):
    import contextlib

    hT = nc.dram_tensor("hT", [H, S], MMD, kind="ExternalInput")
    wqT = nc.dram_tensor("wqT", [H, QH], MMD, kind="ExternalInput")
    wkT = nc.dram_tensor("wkT", [H, DH], MMD, kind="ExternalInput")
    wvT = nc.dram_tensor("wvT", [H, DH], MMD, kind="ExternalInput")
    woT = nc.dram_tensor("woT", [QH, H], MMD, kind="ExternalInput")
    cosT = nc.dram_tensor("cosT", [DH, S], F32, kind="ExternalInput")
    sinT = nc.dram_tensor("sinT", [DH, S], F32, kind="ExternalInput")
    rotT = nc.dram_tensor("rotT", [DH, DH], MMD, kind="ExternalInput")
    ones = nc.dram_tensor("ones", [128, 1], MMD, kind="ExternalInput")
    outT = nc.dram_tensor("outT", [H, S], F32, kind="ExternalOutput")

    const = ctx.enter_context(tc.tile_pool(name="const", bufs=1))

    # Resident weights, [128, kt, m] so lhsT slices are [128, 128-ish]
    wq_sb = const.tile([128, KT, QH], MMD)
    wqT_t = wqT.rearrange("(t p) m -> t p m", p=128)
    for kt in range(KT):
        nc.sync.dma_start(out=wq_sb[:, kt, :], in_=wqT_t[kt])
    wk_sb = const.tile([128, KT, DH], MMD)
    nc.sync.dma_start(out=wk_sb[:], in_=wkT.rearrange("(t p) m -> p t m", p=128))
    wv_sb = const.tile([128, KT, DH], MMD)
    nc.sync.dma_start(out=wv_sb[:], in_=wvT.rearrange("(t p) m -> p t m", p=128))
    wo_sb = const.tile([128, HPC, H], MMD)
    nc.sync.dma_start(out=wo_sb[:], in_=woT.rearrange("(h p) m -> p h m", p=128))
    cos_sb = const.tile([DH, S], F32)
    nc.sync.dma_start(out=cos_sb[:], in_=cosT[:])
    sin_sb = const.tile([DH, S], F32)
    nc.sync.dma_start(out=sin_sb[:], in_=sinT[:])
    rot_sb = const.tile([DH, DH], MMD)
    nc.sync.dma_start(out=rot_sb[:], in_=rotT[:])
    ones_sb = const.tile([128, 1], MMD)
    nc.sync.dma_start(out=ones_sb[:], in_=ones[:])
    ident = const.tile([128, 128], F32)
    make_identity(nc, ident[:])

    # Per-core persistent activations (feature-major)
    q_rope = const.tile([128, HPC, S], MMD)  # rope'd q heads, [d, h, s]
    k_rope = const.tile([128, S], MMD)  # rope'd k, [d, s]
    v_sb = const.tile([128, ST, DH], MMD)  # v, [s-tile part, st, d]

    hT_t = hT.rearrange("(t p) s -> t p s", p=128)

    # ---------------- Phase A: projections ----------------
    with (
        tc.tile_pool(name="hload", bufs=3) as hload,
        tc.tile_pool(name="qps", bufs=1, space="PSUM") as qps,
        tc.tile_pool(name="kvps", bufs=1, space="PSUM") as kvps,
        tc.tile_pool(name="miscps", bufs=1, space="PSUM") as miscps,
        tc.tile_pool(name="evac", bufs=2) as evac,
        tc.tile_pool(name="ropetmp", bufs=2) as ropetmp,
    ):
        for c in range(NCHUNK):
            sl = bass.ts(c, CS)  # this chunk's seq slice
            q_ps = [
                qps.tile([128, CS], F32, tag=f"q{h}", name=f"q_ps{h}", bufs=1)
                for h in range(HPC)
            ]
            k_ps = kvps.tile([128, CS], F32, tag="k")
            vT_ps = kvps.tile([128, CS], F32, tag="v")
            for kt in range(KT):
                h_tile = hload.tile([128, CS], MMD)
                nc.sync.dma_start(out=h_tile[:], in_=hT_t[kt, :, sl])
                mm = dict(start=(kt == 0), stop=(kt == KT - 1))
                for h in range(HPC):
                    nc.tensor.matmul(
                        q_ps[h][:], wq_sb[:, kt, bass.ts(h, DH)], h_tile[:], **mm
                    )
                nc.tensor.matmul(k_ps[:], wk_sb[:, kt, :], h_tile[:], **mm)
                nc.tensor.matmul(vT_ps[:], wv_sb[:, kt, :], h_tile[:], **mm)

            # V: evacuate vT ([d, s]) then PE-transpose into [s, d]
            vT_sb = evac.tile([128, CS], F32, tag="vT")
            nc.scalar.copy(out=vT_sb[:], in_=vT_ps[:])
            vtr_ps = miscps.tile([128, CS], F32, tag="vtr")
            for i in range(CS // 128):
                nc.tensor.transpose(
                    vtr_ps[:, bass.ts(i, 128)], vT_sb[:, bass.ts(i, 128)], ident[:]
                )
            nc.scalar.copy(
                out=v_sb[:, 4 * c : 4 * (c + 1), :].rearrange("p a b -> p (a b)"),
                in_=vtr_ps[:],
            )

            # Q/K: evacuate, rotate via PE, rope via DVE
            for h in range(HPC + 1):
                if h < HPC:
                    src_ps = q_ps[h]
                    dst = q_rope[:, h, sl]
                else:
                    src_ps = k_ps
                    dst = k_rope[:, sl]
                f_sb = evac.tile([128, CS], MMD, tag="f")
                nc.scalar.copy(out=f_sb[:], in_=src_ps[:])
                r_ps = miscps.tile([128, CS], F32, tag="rot")
                nc.tensor.matmul(r_ps[:], rot_sb[:], f_sb[:], start=True, stop=True)
                t1 = ropetmp.tile([128, CS], F32, tag="t1")
                nc.vector.tensor_tensor(t1[:], f_sb[:], cos_sb[:, sl], op=MULT)
                t2 = ropetmp.tile([128, CS], F32, tag="t2")
                nc.vector.tensor_tensor(t2[:], r_ps[:], sin_sb[:, sl], op=MULT)
                nc.vector.tensor_tensor(dst, t1[:], t2[:], op=ADD)

    # ---------------- Phase B+C: attention + o_proj ----------------
    PAIR = 2  # score k-tiles exp'd per ACT op (psum banks per scores tile)
    with (
        tc.tile_pool(name="sps", bufs=2, space="PSUM") as sps,
        tc.tile_pool(name="ops", bufs=2, space="PSUM") as ops,
        tc.tile_pool(name="dps", bufs=1, space="PSUM") as dps,
        tc.tile_pool(name="cps", bufs=1, space="PSUM") as cps,
        tc.tile_pool(name="expp", bufs=3) as expp,
        tc.tile_pool(name="opool", bufs=2) as opool,
        tc.tile_pool(name="small", bufs=2) as small,
        tc.tile_pool(name="outev", bufs=3) as outev,
        tc.tile_pool(name="drb", bufs=2, space="DRAM") as drb,
    ):
        for c in range(NCHUNK):
            sl = bass.ts(c, CS)
            o_chunk = opool.tile([128, HPC, CS], MMD)
            for h in range(HPC):
                o_ps = ops.tile([128, CS], F32)
                dacc = small.tile([128, CS], MMD, tag="dacc")
                for kp in range(ST // PAIR):
                    s_ps = sps.tile([128, PAIR * CS], F32)
                    for j in range(PAIR):
                        kt = kp * PAIR + j
                        nc.tensor.matmul(
                            s_ps[:, bass.ts(j, CS)],
                            k_rope[:, bass.ts(kt, 128)],
                            q_rope[:, h, sl],
                            start=True,
                            stop=True,
                        )
                    e_sb = expp.tile([128, PAIR * CS], MMD)
                    nc.scalar.activation(e_sb[:], s_ps[:], Exp, scale=SCALE)
                    for j in range(PAIR):
                        kt = kp * PAIR + j
                        mm = dict(start=(kt == 0), stop=(kt == ST - 1))
                        nc.tensor.matmul(
                            o_ps[:], v_sb[:, kt, :], e_sb[:, bass.ts(j, CS)], **mm
                        )
                    # denominator partials on DVE (bf16 acc, ~3e-4 rel err)
                    if kp == 0:
                        nc.vector.tensor_tensor(
                            dacc[:], e_sb[:, 0:CS], e_sb[:, CS : 2 * CS], op=ADD
                        )
                    else:
                        psum_t = small.tile(
                            [128, CS], MMD, tag="dtmp", name=f"dtmp{kp}"
                        )
                        nc.vector.tensor_tensor(
                            psum_t[:], e_sb[:, 0:CS], e_sb[:, CS : 2 * CS], op=ADD
                        )
                        nc.vector.tensor_tensor(dacc[:], dacc[:], psum_t[:], op=ADD)
                # reduce denom over partitions via ones-matmul, then normalize
                d_ps = dps.tile([1, CS], F32)
                nc.tensor.matmul(d_ps[:], ones_sb[:], dacc[:], start=True, stop=True)
                recip = small.tile([1, CS], F32, tag="recip")
                nc.vector.reciprocal_approx_fast(recip[:], d_ps[:])
                r_dram = drb.tile([1, CS], F32)
                nc.sync.dma_start(out=r_dram[:], in_=recip[:])
                recip_bc = small.tile([128, CS], F32, tag="rbc")
                nc.sync.dma_start(
                    out=recip_bc[:], in_=r_dram[:].partition_broadcast(128)
                )
                nc.vector.tensor_tensor(
                    o_chunk[:, h, :], o_ps[:], recip_bc[:], op=MULT
                )
            # o_proj partial for this chunk
            for mt in range(KT):
                c_ps = cps.tile([128, CS], F32)
                for h in range(HPC):
                    nc.tensor.matmul(
                        c_ps[:],
                        wo_sb[:, h, bass.ts(mt, 128)],
                        o_chunk[:, h, :],
                        start=(h == 0),
                        stop=(h == HPC - 1),
                    )
                o_ev = outev.tile([128, CS], F32)
                nc.scalar.copy(out=o_ev[:], in_=c_ps[:])
                nc.sync.dma_start(out=outT[bass.ts(mt, 128), sl], in_=o_ev[:])


def build():
    nc = bacc.Bacc("TRN2", target_bir_lowering=False)
    import contextlib

    with tile.TileContext(nc) as tc:
        with contextlib.ExitStack() as ctx:
            _emit(nc, tc, ctx)
    nc.compile()
    return nc


_NC = None


def _get_nc():
    global _NC
    if _NC is None:
        _NC = build()
    return _NC


def _host_tables():
    inv = 1.0 / (ROPE_THETA ** (np.arange(0, DH, 2, dtype=np.float64) / DH))
    t = np.arange(S, dtype=np.float64)
    freqs = np.outer(t, inv)  # [S, 64]
    emb = np.concatenate([freqs, freqs], axis=1)  # [S, 128]
    cosT = np.ascontiguousarray(np.cos(emb).T.astype(np.float32))  # [128, S]
    sinT = np.ascontiguousarray(np.sin(emb).T.astype(np.float32))
    # rot[d,:] selects rotate_half: rot @ q = concat(-q_hi, q_lo)
    half = DH // 2
    rot = np.zeros((DH, DH), np.float32)
    for d in range(half):
        rot[d, d + half] = -1.0
        rot[d + half, d] = 1.0
    rotT = np.ascontiguousarray(rot.T)
    return cosT, sinT, rotT


LAST_EXEC_TIME_NS = None
LAST_TRACE = None


def _mmd_np(a):
    if COMPUTE_DT == "bf16":
        import ml_dtypes

        return np.ascontiguousarray(a.astype(ml_dtypes.bfloat16))
    return np.ascontiguousarray(a.astype(np.float32))


def kernel(hidden_states, attention_mask, Wq, Wk, Wv, Wo):
    global LAST_EXEC_TIME_NS, LAST_TRACE
    hidden_states = np.asarray(hidden_states, dtype=np.float32)
    Wq = np.asarray(Wq, dtype=np.float32)
    Wk = np.asarray(Wk, dtype=np.float32)
    Wv = np.asarray(Wv, dtype=np.float32)
    Wo = np.asarray(Wo, dtype=np.float32)

    nc = _get_nc()
    cosT, sinT, rotT = _host_tables()
    ones = np.ones((128, 1), np.float32)

    hTs = [_mmd_np(hidden_states[b].T) for b in range(B)]
    in_maps = []
    for core in range(NCORES):
        b, g = divmod(core, TPG)
        qsl = slice(g * QH, (g + 1) * QH)
        ksl = slice(g * DH, (g + 1) * DH)
        in_maps.append(
            {
                "hT": hTs[b],
                "wqT": _mmd_np(Wq[qsl].T),
                "wkT": _mmd_np(Wk[ksl].T),
                "wvT": _mmd_np(Wv[ksl].T),
                "woT": _mmd_np(Wo[:, qsl].T),
                "cosT": cosT,
                "sinT": sinT,
                "rotT": _mmd_np(rotT),
                "ones": _mmd_np(ones),
            }
        )

    trace = bool(os.environ.get("BASS_KERNEL_TRACE"))
    kw = {}
    if trace:
        kw = dict(trace=True, trace_cores=list(range(NCORES)))
    res = run_bass_kernel_spmd(nc, in_maps, core_ids=list(range(NCORES)), **kw)
    LAST_EXEC_TIME_NS = res.exec_time_ns
    LAST_TRACE = res.instructions_and_trace[1] if res.instructions_and_trace else None

    out = np.zeros((B, H, S), np.float32)
    for core in range(NCORES):
        out[core // TPG] += res.results[core]["outT"]
    return np.ascontiguousarray(out.transpose(0, 2, 1))
